# revision 12
# baseline (speedup 1.0000x reference)
"""GraphShiftOperator on 8 Trainium2 NeuronCores (raw Bass, explicit sync).

reference:
    out_deg = A.sum(1); in_deg = A.sum(0)
    forward = A.T * (1/(out_deg+eps))[None, :]   # = (diag(1/out_deg) @ A).T
    reverse = A  * (1/(in_deg+eps))[None, :]

Sharding: rows of A across 8 cores (1024 rows each).
  - out_deg local (row sums);  in_deg needs one 32KB AllReduce (col sums).
  - core s returns:
      fwd_scaled = A_s * d_out_inv[:, None]   (columns of `forward`;
                   host assembles forward = vstack(...).T, a free view)
      rev        = A_s * d_in_inv[None, :]    (rows of `reverse`)

The tolerance gate (2e-2) leaves a lot of precision headroom, so A is
staged to fp16 on the host and both outputs are produced in fp16 and
upcast on the host. That makes the whole 16MB per-core shard fit in
SBUF (one HBM read instead of two) and halves the output writes:
48MB of HBM traffic per core instead of 128MB for the all-f32
double-read design.

Engine split per core:
  SP(sync)   8 A-tile loads (2MB fp16 each), then the 8 fwd stores
  ACT        fwd multiplies (A_t * d_out_inv_t, per-partition scalar),
             then the 8 rev stores
  DVE        row sums + reciprocals, colsum copies, global recip,
             rev multiplies (in place on the resident A tiles)
  PE         column sums (A_chunk stationary @ ones, accumulated in
             PSUM) + transpose to the collective layout
  GPSIMD     identity build, collective + bounce DMAs, d_in broadcast

`_build(iters=K)` repeats the whole workload K times inside one NEFF
(reloading A from DRAM, redoing the collective) so test.py can measure
per-iteration HW time as (T(K) - T(1)) / (K-1) with the host/tunnel
dispatch constant cancelled. iters=1 is the production program.
`sim_local=True` replaces the collective with a local bounce so the
program can run under the single-core TimelineSim cost model.
"""

import sys

sys.path.insert(0, "/opt/trn_rl_repo")

from contextlib import ExitStack

import numpy as np

import concourse.bass as bass
from concourse import mybir
from concourse.bass_utils import run_bass_kernel_spmd

N = 8192
N_CORES = 8
ROWS = N // N_CORES          # 1024 rows per core
P = 128                      # partitions
NT = ROWS // P               # 8 row-tiles per core
NCC = N // P                 # 64 column chunks of 128
EPS = 1e-8
F32 = mybir.dt.float32
F16 = mybir.dt.float16

_cache = {}


def _build(iters: int = 1, sim_local: bool = False):
    nc = bass.Bass(num_devices=N_CORES)

    a_in = nc.dram_tensor("a_shard", [ROWS, N], F16, kind="ExternalInput")
    fwd_out = nc.dram_tensor("fwd_scaled", [ROWS, N], F16, kind="ExternalOutput")
    rev_out = nc.dram_tensor("rev", [ROWS, N], F16, kind="ExternalOutput")
    cc_in = nc.dram_tensor("cc_in", [NCC, P], F32)
    cc_out = nc.dram_tensor("cc_out", [NCC, P], F32)
    dri = nc.dram_tensor("dri", [NCC, P], F16)

    ctx = ExitStack()
    with ctx:
        sem = lambda name: ctx.enter_context(nc.semaphore(name))
        li = sem("li")      # A-tile loads            (+16 each)
        on = sem("on")      # ones memset done        (+1, once)
        ps0 = sem("ps0")    # colsum PSUM zeroed      (+1 per iter)
        dv1 = sem("dv1")    # doi_t ready             (+1 per row-tile)
        am = sem("am")      # ACT fwd multiply done   (+1 per row-tile)
        fo = sem("fo")      # fwd store done          (+16 per row-tile)
        pe = sem("pe")      # colsum matmuls done     (+1 per iter)
        idt = sem("idt")    # identity ready          (+1, once)
        cp1 = sem("cp1")    # colsum psum->sbuf copy  (+1 per iter)
        tr = sem("tr")      # PE transpose done       (+1 per iter)
        cp = sem("cp")      # transpose psum->sbuf    (+1 per iter)
        cci = sem("cci")    # cc_in bounce DMA done   (+16 per iter)
        cc = sem("cc")      # collective done         (+1 per iter)
        gld = sem("gld")    # global colsums loaded   (+16 per iter)
        rdy = sem("rdy")    # fp16 recip ready        (+1 per iter)
        drid = sem("drid")  # dri store done          (+16 per iter)
        dinb = sem("dinb")  # d_in broadcast done     (+16 per iter)
        dv2 = sem("dv2")    # rev multiply done       (+1 per row-tile)
        ro = sem("ro")      # rev store done          (+16 per row-tile)

        sbuf = lambda name, shape, dt: ctx.enter_context(
            nc.sbuf_tensor(name, shape, dt)
        )
        a_sb = [sbuf(f"a{i}", [P, N], F16) for i in range(NT)]  # whole shard
        f_sb = [sbuf(f"f{i}", [P, N], F16) for i in range(3)]   # fwd staging
        din = sbuf("din", [P, N], F16)
        ones = sbuf("ones", [P, 1], F16)
        rs = sbuf("rs", [P, 1], F32)
        doi = [sbuf(f"doi{i}", [P, 1], F32) for i in range(NT)]
        ident = sbuf("ident", [P, P], F32)
        cs_sb = sbuf("cs_sb", [P, NCC], F32)
        csT = sbuf("csT", [NCC, P], F32)
        gs = sbuf("gs", [NCC, P], F32)
        gr16 = sbuf("gr16", [NCC, P], F16)

        cs_ps = ctx.enter_context(nc.psum_tensor("cs_ps", [P, NCC], F32))
        tr_ps = ctx.enter_context(nc.psum_tensor("tr_ps", [NCC, P], F32))

        with nc.allow_low_precision("fp16 staging is well inside the 2e-2 gate"):
            with nc.Block() as block:

                @block.sync
                def _(sync):
                    for i in range(iters):
                        for t in range(NT):
                            if i > 0:
                                # a_sb slot freed by prev iter's rev store
                                sync.wait_ge(ro, 16 * ((i - 1) * NT + t + 1))
                            sync.dma_start(
                                out=a_sb[t][:], in_=a_in[t * P : (t + 1) * P, :]
                            ).then_inc(li, 16)
                        for t in range(NT):
                            g = i * NT + t
                            sync.wait_ge(am, g + 1)
                            sync.dma_start(
                                out=fwd_out[t * P : (t + 1) * P, :],
                                in_=f_sb[g % 3][:],
                            ).then_inc(fo, 16)
                    # the NEFF must not retire before the last output DMAs
                    # land — nothing else waits on the tail stores
                    sync.wait_ge(fo, 16 * iters * NT)
                    sync.wait_ge(ro, 16 * iters * NT)

                @block.scalar
                def _(scalar):
                    for i in range(iters):
                        for t in range(NT):
                            g = i * NT + t
                            scalar.wait_ge(dv1, g + 1)  # doi_t ready (=> loaded)
                            if g >= 3:
                                scalar.wait_ge(fo, 16 * (g - 2))  # slot free
                            scalar.mul(f_sb[g % 3][:], a_sb[t][:], doi[t][:])
                            scalar.drain().then_inc(am, 1)
                        for t in range(NT):
                            scalar.wait_ge(dv2, i * NT + t + 1)
                            scalar.dma_start(
                                out=rev_out[t * P : (t + 1) * P, :], in_=a_sb[t][:]
                            ).then_inc(ro, 16)

                @block.vector
                def _(vector):
                    # NOTE: DVE results are not visible (even to DVE itself)
                    # until an explicit drain; raw bass must do it by hand.
                    vector.memset(ones[:], 1.0)
                    vector.drain().then_inc(on, 1)
                    for i in range(iters):
                        # matmul start=True resets the WHOLE PSUM bank, so the
                        # interleaved accumulation below runs start=False onto
                        # a pre-zeroed bank instead.
                        vector.memset(cs_ps[:], 0.0)
                        vector.drain().then_inc(ps0, 1)
                        for t in range(NT):
                            vector.wait_ge(li, 16 * (i * NT + t + 1))
                            vector.reduce_sum(
                                out=rs[:], in_=a_sb[t][:], axis=mybir.AxisListType.X
                            )
                            vector.drain()
                            vector.tensor_scalar_add(rs[:], rs[:], EPS)
                            vector.drain()
                            if i > 0:
                                # doi_t consumed by prev iter's fwd multiply
                                vector.wait_ge(am, (i - 1) * NT + t + 1)
                            vector.reciprocal(doi[t][:], rs[:])
                            vector.drain().then_inc(dv1, 1)
                        # local colsums: psum -> sbuf, PE-transpose into the
                        # [chunk, lane] layout the collective + broadcast want
                        vector.wait_ge(pe, i + 1)
                        vector.tensor_copy(cs_sb[:], cs_ps[:])
                        vector.drain().then_inc(cp1, 1)
                        vector.wait_ge(tr, i + 1)
                        if i > 0:
                            # csT consumed by prev iter's cc_in bounce
                            vector.wait_ge(cci, 16 * i)
                        vector.tensor_copy(csT[:], tr_ps[:])
                        vector.drain().then_inc(cp, 1)
                        # d_in_inv = fp16(1/(in_deg+eps)) on the compact [64,128]
                        vector.wait_ge(gld, 16 * (i + 1))
                        vector.tensor_scalar_add(gs[:], gs[:], EPS)
                        vector.drain()
                        vector.reciprocal(gs[:], gs[:])
                        vector.drain()
                        if i > 0:
                            # gr16 consumed by prev iter's dri store
                            vector.wait_ge(drid, 16 * i)
                        vector.tensor_copy(gr16[:], gs[:])
                        vector.drain().then_inc(rdy, 1)
                        for t in range(NT):
                            if t == 0:
                                vector.wait_ge(dinb, 16 * (i + 1))
                            vector.wait_ge(am, i * NT + t + 1)  # ACT read A_t
                            vector.tensor_mul(a_sb[t][:], a_sb[t][:], din[:])
                            vector.drain().then_inc(dv2, 1)

                @block.tensor
                def _(tensor):
                    tensor.wait_ge(on, 1)
                    for i in range(iters):
                        tensor.wait_ge(ps0, i + 1)
                        for t in range(NT):
                            tensor.wait_ge(li, 16 * (i * NT + t + 1))
                            for c in range(NCC):
                                mm = tensor.matmul(
                                    cs_ps[:, c : c + 1],
                                    a_sb[t][:, c * P : (c + 1) * P],
                                    ones[:],
                                    start=False,
                                    stop=(t == NT - 1 and c == NCC - 1),
                                    skip_group_check=True,
                                )
                        mm.then_inc(pe, 1)
                        if i == 0:
                            tensor.wait_ge(idt, 1)
                        else:
                            # tr_ps consumed by prev iter's csT copy
                            tensor.wait_ge(cp, i)
                        tensor.wait_ge(cp1, i + 1)
                        tensor.transpose(tr_ps[:], cs_sb[:], ident[:]).then_inc(
                            tr, 1
                        )

                @block.gpsimd
                def _(gpsimd):
                    gpsimd.memset(ident[:], 0.0)
                    gpsimd.affine_select(
                        out=ident[:],
                        in_=ident[:],
                        compare_op=mybir.AluOpType.not_equal,
                        fill=1.0,
                        base=0,
                        pattern=[[-1, P]],
                        channel_multiplier=1,
                    ).then_inc(idt, 1)
                    for i in range(iters):
                        gpsimd.wait_ge(cp, i + 1)
                        if i > 0:
                            # cc_in consumed by prev iter's collective/load
                            gpsimd.wait_ge(gld, 16 * i) if sim_local else \
                                gpsimd.wait_ge(cc, i)
                        gpsimd.dma_start(out=cc_in[:], in_=csT[:]).then_inc(cci, 16)
                        gpsimd.wait_ge(cci, 16 * (i + 1))
                        if i > 0:
                            # cc_out consumed by prev iter's gs load
                            gpsimd.wait_ge(gld, 16 * i)
                        if sim_local:
                            gpsimd.dma_start(out=gs[:], in_=cc_in[:]).then_inc(
                                gld, 16
                            )
                        else:
                            gpsimd.collective_compute(
                                "AllReduce",
                                mybir.AluOpType.add,
                                replica_groups=[list(range(N_CORES))],
                                ins=[cc_in[:]],
                                outs=[cc_out[:]],
                            ).then_inc(cc, 1)
                            gpsimd.wait_ge(cc, i + 1)
                            gpsimd.dma_start(out=gs[:], in_=cc_out[:]).then_inc(
                                gld, 16
                            )
                        gpsimd.wait_ge(rdy, i + 1)
                        if i > 0:
                            # dri consumed by prev iter's broadcast
                            gpsimd.wait_ge(dinb, 16 * i)
                        gpsimd.dma_start(out=dri[:], in_=gr16[:]).then_inc(drid, 16)
                        gpsimd.wait_ge(drid, 16 * (i + 1))
                        if i > 0:
                            # din consumed by prev iter's rev multiplies
                            gpsimd.wait_ge(dv2, i * NT)
                        gpsimd.dma_start(
                            out=din[:],
                            in_=bass.AP(dri, 0, [[0, P], [1, N]]),
                        ).then_inc(dinb, 16)

    return nc


def kernel(adjacency_matrix: np.ndarray, _trace=False, _trace_kwargs=None):
    a = np.asarray(adjacency_matrix)
    assert a.shape == (N, N)
    a16 = np.ascontiguousarray(a, dtype=np.float16)

    if "nc" not in _cache:
        _cache["nc"] = _build()
    nc = _cache["nc"]

    in_maps = [
        {"a_shard": a16[s * ROWS : (s + 1) * ROWS, :]} for s in range(N_CORES)
    ]
    kw = {}
    if _trace:
        kw = dict(trace=True, **(_trace_kwargs or {}))
    res = run_bass_kernel_spmd(nc, in_maps, list(range(N_CORES)), **kw)

    scaled = np.concatenate([r["fwd_scaled"] for r in res.results], axis=0)
    reverse = np.concatenate([r["rev"] for r in res.results], axis=0)
    forward = scaled.astype(np.float32).T
    reverse = reverse.astype(np.float32)
    if _trace:
        return (forward, reverse), res
    return forward, reverse


# revision 17
# speedup vs baseline: 7.1232x; 7.1232x over previous
"""GraphShiftOperator on 8 Trainium2 NeuronCores (raw Bass, explicit sync).

reference:
    out_deg = A.sum(1); in_deg = A.sum(0)
    forward = A.T * (1/(out_deg+eps))[None, :]   # = (diag(1/out_deg) @ A).T
    reverse = A  * (1/(in_deg+eps))[None, :]

Sharding: communication-free hybrid. Core s gets BOTH
  - the row stripe    A[s*1024:(s+1)*1024, :]   (16MB fp16), and
  - the column stripe A[:, s*1024:(s+1)*1024]   (16MB fp16).
It computes
  - fwd_scaled = A_rows * d_out_inv[:, None]  from purely local row sums
    (host assembles forward = vstack(...).T, a free view), and
  - rev_cols   = A_cols * d_in_inv[None, :]   from purely local column
    sums of its own column stripe.
No cross-core exchange at all: a measured bisect showed a single 32KB
AllReduce through this runtime costs ~5.4ms — 25x the whole kernel — so
trading one extra 16MB stripe read (~46us) for the collective wins big.

The tolerance gate (2e-2) leaves a lot of precision headroom, so A is
staged to fp16 on the host and both outputs are produced in fp16 and
upcast on the host (~1.2e-3 max rel err). Per-core HBM traffic:
32MB in + 32MB out ~= 180us at ~360GB/s, the memory roofline.

Engine split per core:
  SP(sync)   16 column-stripe loads (1MB packed tiles)
  GPSIMD     8 row-stripe loads (2MB tiles), d_in bounce + broadcast,
             identity build
  DVE        row sums + reciprocals, colsum copies + global recip,
             rev multiplies (in place on the resident column stripe)
  ACT        fwd multiplies (in place) + fwd stores, rev stores
  PE         column sums (A_chunk stationary @ ones -> [128,8] PSUM)
             + transpose of the 1024-vector to broadcast layout

`_build(iters=K)` repeats the whole workload K times inside one NEFF
(reloading both stripes from DRAM each iteration) so test.py can
measure per-iteration HW time as (T(K) - T(1)) / (K-1) with the
host/tunnel dispatch constant cancelled. iters=1 is the production
program, and it is also directly simulable with TimelineSim.
"""

import sys

sys.path.insert(0, "/opt/trn_rl_repo")

from contextlib import ExitStack

import numpy as np

import concourse.bass as bass
from concourse import mybir
from concourse.bass_utils import run_bass_kernel_spmd

N = 8192
N_CORES = 8
SC = N // N_CORES            # 1024 stripe rows/cols per core
P = 128                      # partitions
NT = SC // P                 # 8 row-tiles per core
CTT = 16                     # column-stripe packed tiles
SEG = N // CTT // P          # 4 row-segments packed per column tile
EPS = 1e-8
F32 = mybir.dt.float32
F16 = mybir.dt.float16

_cache = {}


def _build(iters: int = 1):
    nc = bass.Bass(num_devices=N_CORES)

    a_rows = nc.dram_tensor("a_rows", [SC, N], F16, kind="ExternalInput")
    a_cols = nc.dram_tensor("a_cols", [N, SC], F16, kind="ExternalInput")
    fwd_out = nc.dram_tensor("fwd_scaled", [SC, N], F16, kind="ExternalOutput")
    rev_out = nc.dram_tensor("rev_cols", [N, SC], F16, kind="ExternalOutput")
    dri = nc.dram_tensor("dri", [SEG * 2, P], F16)  # d_in_inv bounce, [8,128]

    # packed column tile k covers stripe rows [k*512, (k+1)*512), laid out as
    # 4 segments of 128 rows side by side in the free dim:
    #   c_sb[k][p, s*1024 + j] = A[k*512 + s*128 + p, cols_s[j]]
    def col_ap(dram, k):
        return bass.AP(
            dram, k * SEG * P * SC, [[SC, P], [P * SC, SEG], [1, SC]]
        )

    ctx = ExitStack()
    with ctx:
        sem = lambda name: ctx.enter_context(nc.semaphore(name))
        cl = sem("cl")      # col-tile loads          (+16 each)
        rl = sem("rl")      # row-tile loads          (+16 each)
        on = sem("on")      # ones memset done        (+1, once)
        ps0 = sem("ps0")    # colsum PSUM zeroed      (+1 per iter)
        dv1 = sem("dv1")    # doi ready               (+1 per row-tile)
        am = sem("am")      # ACT fwd multiply done   (+1 per row-tile)
        fo = sem("fo")      # fwd store done          (+16 per row-tile)
        pe = sem("pe")      # colsum matmuls done     (+1 per iter)
        idt = sem("idt")    # identity ready          (+1, once)
        cp1 = sem("cp1")    # colsum psum->sbuf copy  (+1 per iter)
        tr = sem("tr")      # PE transpose done       (+1 per iter)
        cp2 = sem("cp2")    # transpose psum->sbuf    (+1 per iter)
        rdy = sem("rdy")    # fp16 recip ready        (+1 per iter)
        drid = sem("drid")  # dri store done          (+16 per iter)
        dinb = sem("dinb")  # d_in broadcast done     (+16 per iter)
        dv2 = sem("dv2")    # rev multiply done       (+1 per col-tile)
        ro = sem("ro")      # rev store done          (+16 per col-tile)

        sbuf = lambda name, shape, dt: ctx.enter_context(
            nc.sbuf_tensor(name, shape, dt)
        )
        c_sb = [sbuf(f"c{i}", [P, SEG * SC], F16) for i in range(CTT)]  # 16MB
        r_sb = [sbuf(f"r{i}", [P, N], F16) for i in range(4)]           # 8MB
        din = sbuf("din", [P, SC], F16)
        ones = sbuf("ones", [P, 1], F16)
        rs = sbuf("rs", [P, 1], F32)
        doi = [sbuf(f"doi{i}", [P, 1], F32) for i in range(4)]
        ident = sbuf("ident", [P, P], F32)
        cs8 = sbuf("cs8", [P, SEG * 2], F32)
        trc = sbuf("trc", [SEG * 2, P], F32)
        gr16 = sbuf("gr16", [SEG * 2, P], F16)

        cs_ps = ctx.enter_context(nc.psum_tensor("cs_ps", [P, SEG * 2], F32))
        tr_ps = ctx.enter_context(nc.psum_tensor("tr_ps", [SEG * 2, P], F32))

        with nc.allow_low_precision("fp16 staging is well inside the 2e-2 gate"):
            with nc.Block() as block:

                # DMA-completion semaphores tick +1 per engine-slice (16 per
                # transfer), and slices of several in-flight transfers
                # complete interleaved — `sem >= 16*(k+1)` does NOT mean
                # transfers 0..k are done, only that k+1 transfers' worth of
                # slices landed. Waits on multi-transfer semaphores therefore
                # add one transfer of slack (capped at the count issued
                # unconditionally), which covers the per-engine skew.
                @block.sync
                def _(sync):
                    for i in range(iters):
                        for k in range(CTT):
                            if i > 0:
                                # c_sb slot freed by prev iter's rev store
                                sync.wait_ge(
                                    ro,
                                    min(
                                        16 * ((i - 1) * CTT + k + 1) + 16,
                                        16 * i * CTT,
                                    ),
                                )
                            sync.dma_start(
                                out=c_sb[k][:], in_=col_ap(a_cols, k)
                            ).then_inc(cl, 16)
                    # the NEFF must not retire before the last output DMAs land
                    sync.wait_ge(fo, 16 * NT * iters)
                    sync.wait_ge(ro, 16 * CTT * iters)

                @block.scalar
                def _(scalar):
                    for i in range(iters):
                        for t in range(NT):
                            g = i * NT + t
                            scalar.wait_ge(dv1, g + 1)  # doi ready => loaded
                            scalar.mul(r_sb[g % 4][:], r_sb[g % 4][:], doi[g % 4][:])
                            scalar.drain().then_inc(am, 1)
                            scalar.dma_start(
                                out=fwd_out[t * P : (t + 1) * P, :],
                                in_=r_sb[g % 4][:],
                            ).then_inc(fo, 16)
                        for k in range(CTT):
                            scalar.wait_ge(dv2, i * CTT + k + 1)
                            scalar.dma_start(
                                out=col_ap(rev_out, k), in_=c_sb[k][:]
                            ).then_inc(ro, 16)

                @block.vector
                def _(vector):
                    # NOTE: DVE results are not visible (even to DVE itself)
                    # until an explicit drain; raw bass must do it by hand.
                    vector.memset(ones[:], 1.0)
                    vector.drain().then_inc(on, 1)
                    for i in range(iters):
                        # matmul start=True resets the WHOLE PSUM bank, so the
                        # interleaved accumulation below runs start=False onto
                        # a pre-zeroed bank instead.
                        vector.memset(cs_ps[:], 0.0)
                        vector.drain().then_inc(ps0, 1)
                        for t in range(NT):
                            g = i * NT + t
                            vector.wait_ge(
                                rl, min(16 * (g + 1) + 16, 16 * NT * (i + 1))
                            )
                            vector.reduce_sum(
                                out=rs[:], in_=r_sb[g % 4][:],
                                axis=mybir.AxisListType.X,
                            )
                            vector.drain()
                            vector.tensor_scalar_add(rs[:], rs[:], EPS)
                            vector.drain()
                            if g >= 4:
                                # doi slot consumed by fwd multiply g-4
                                vector.wait_ge(am, g - 3)
                            vector.reciprocal(doi[g % 4][:], rs[:])
                            vector.drain().then_inc(dv1, 1)
                        # local stripe colsums -> [8,128] broadcast layout
                        vector.wait_ge(pe, i + 1)
                        vector.tensor_copy(cs8[:], cs_ps[:])
                        vector.drain().then_inc(cp1, 1)
                        vector.wait_ge(tr, i + 1)
                        vector.tensor_copy(trc[:], tr_ps[:])
                        vector.drain().then_inc(cp2, 1)
                        vector.tensor_scalar_add(trc[:], trc[:], EPS)
                        vector.drain()
                        vector.reciprocal(trc[:], trc[:])
                        vector.drain()
                        if i > 0:
                            # gr16 consumed by prev iter's dri store
                            vector.wait_ge(drid, 16 * i)
                        vector.tensor_copy(gr16[:], trc[:])
                        vector.drain().then_inc(rdy, 1)
                        for k in range(CTT):
                            if k == 0:
                                vector.wait_ge(dinb, 16 * (i + 1))
                            for s in range(SEG):
                                vector.tensor_mul(
                                    c_sb[k][:, s * SC : (s + 1) * SC],
                                    c_sb[k][:, s * SC : (s + 1) * SC],
                                    din[:],
                                )
                            vector.drain().then_inc(dv2, 1)

                @block.tensor
                def _(tensor):
                    tensor.wait_ge(on, 1)
                    for i in range(iters):
                        tensor.wait_ge(ps0, i + 1)
                        for k in range(CTT):
                            tensor.wait_ge(
                                cl,
                                min(
                                    16 * (i * CTT + k + 1) + 32,
                                    16 * (i + 1) * CTT,
                                ),
                            )
                            for s in range(SEG):
                                for c in range(SEG * 2):
                                    mm = tensor.matmul(
                                        cs_ps[:, c : c + 1],
                                        c_sb[k][
                                            :,
                                            s * SC + c * P : s * SC + (c + 1) * P,
                                        ],
                                        ones[:],
                                        start=False,
                                        stop=(
                                            k == CTT - 1
                                            and s == SEG - 1
                                            and c == SEG * 2 - 1
                                        ),
                                        skip_group_check=True,
                                    )
                        mm.then_inc(pe, 1)
                        if i == 0:
                            tensor.wait_ge(idt, 1)
                        else:
                            # tr_ps consumed by prev iter's trc copy
                            tensor.wait_ge(cp2, i)
                        tensor.wait_ge(cp1, i + 1)
                        tensor.transpose(tr_ps[:], cs8[:], ident[:]).then_inc(
                            tr, 1
                        )

                @block.gpsimd
                def _(gpsimd):
                    gpsimd.memset(ident[:], 0.0)
                    gpsimd.affine_select(
                        out=ident[:],
                        in_=ident[:],
                        compare_op=mybir.AluOpType.not_equal,
                        fill=1.0,
                        base=0,
                        pattern=[[-1, P]],
                        channel_multiplier=1,
                    ).then_inc(idt, 1)
                    for i in range(iters):
                        for t in range(NT):
                            g = i * NT + t
                            if g >= 4:
                                # r_sb slot freed by fwd store g-4
                                gpsimd.wait_ge(fo, 16 * (g - 2))
                            gpsimd.dma_start(
                                out=r_sb[g % 4][:],
                                in_=a_rows[t * P : (t + 1) * P, :],
                            ).then_inc(rl, 16)
                        gpsimd.wait_ge(rdy, i + 1)
                        if i > 0:
                            # dri consumed by prev iter's broadcast
                            gpsimd.wait_ge(dinb, 16 * i)
                        gpsimd.dma_start(out=dri[:], in_=gr16[:]).then_inc(
                            drid, 16
                        )
                        gpsimd.wait_ge(drid, 16 * (i + 1))
                        if i > 0:
                            # din consumed by prev iter's rev multiplies
                            gpsimd.wait_ge(dv2, i * CTT)
                        gpsimd.dma_start(
                            out=din[:],
                            in_=bass.AP(dri, 0, [[0, P], [1, SC]]),
                        ).then_inc(dinb, 16)

    return nc


def kernel(adjacency_matrix: np.ndarray, _trace=False, _trace_kwargs=None):
    a = np.asarray(adjacency_matrix)
    assert a.shape == (N, N)
    a16 = np.ascontiguousarray(a, dtype=np.float16)

    if "nc" not in _cache:
        _cache["nc"] = _build()
    nc = _cache["nc"]

    in_maps = [
        {
            "a_rows": a16[s * SC : (s + 1) * SC, :],
            "a_cols": np.ascontiguousarray(a16[:, s * SC : (s + 1) * SC]),
        }
        for s in range(N_CORES)
    ]
    kw = {}
    if _trace:
        kw = dict(trace=True, **(_trace_kwargs or {}))
    res = run_bass_kernel_spmd(nc, in_maps, list(range(N_CORES)), **kw)

    scaled = np.concatenate([r["fwd_scaled"] for r in res.results], axis=0)
    reverse = np.concatenate([r["rev_cols"] for r in res.results], axis=1)
    forward = scaled.astype(np.float32).T
    reverse = reverse.astype(np.float32)
    if _trace:
        return (forward, reverse), res
    return forward, reverse
